# revision 1
# baseline (speedup 1.0000x reference)
"""GCN context-paper kernel for 8 trn2 NeuronCores (SPMD via bass/Tile).

Model (see reference): proj+LN -> 3x GCNConv(+self loops, sym-norm) with
GELU -> concat(4 hops) -> MLP(GELU) -> LN.

Sharding: nodes partitioned across 8 cores (2500/core, padded to 2560).
Per hop: each core computes Y = h @ W for its nodes, AllGathers Y (bf16),
then builds its nodes' aggregation with indirect row-gathers of Y plus
one-hot matmuls on the tensor engine (edge weights folded into the
one-hot values, self-loops folded into the edge list).

Layout strategy: activations are kept feature-major ("ct" tiles,
[128 feat, 2560 nodes]) which the scatter matmul produces directly and
all lhsT uses consume directly; only the proj output needs PE transposes.

DMA discipline: every DMA-queue instruction must end up with at most ONE
semaphore wait (hardware struct limit). Hence: DMA destinations in SBUF
are either fresh tiles or have engine-op (not DMA) prior writers; DMA
sources are external inputs or covered by dummy lane-warming DMAs
(collective output).
"""

import numpy as np
import ml_dtypes

import concourse.bass as bass
import concourse.bacc as bacc
import concourse.mybir as mybir
import concourse.tile as tile
from concourse.bass_utils import run_bass_kernel_spmd
from concourse.masks import make_identity

# problem constants (hardcoded per contract)
N, E, IN_F, H, HOPS = 20000, 100000, 1536, 768, 3
LN_EPS = 1e-5
NCORES = 8
NLOC = N // NCORES            # 2500 real nodes per core
P = 128
MT = 20                       # node tiles per core
NPAD = MT * P                 # 2560 padded nodes per core
HK = H // P                   # 6 feature tiles
INK = IN_F // P               # 12
CK = (HOPS + 1) * H // P      # 24 cat feature tiles
NSL = ((0, 512), (512, 256))  # N-dim slices for 768-wide outputs
OB = 8                        # chunks per one-hot load
GRP = 8                       # edge chunks per dma_gather

F32 = mybir.dt.float32
BF16 = mybir.dt.bfloat16
I32 = mybir.dt.int32
BF = ml_dtypes.bfloat16


# ---------------------------------------------------------------- host prep

def _prep(edge_index):
    """Host preprocessing: normalization, edge sorting, per-core chunk
    tables (gather indices + one-hot weight blocks)."""
    src = np.asarray(edge_index[0], dtype=np.int64)
    dst = np.asarray(edge_index[1], dtype=np.int64)
    deg = np.bincount(dst, minlength=N).astype(np.float64) + 1.0
    dis = 1.0 / np.sqrt(deg)

    # real edges only; self loops become a dedicated per-tile identity chunk
    alls, alld = src, dst
    w = (dis[alls] * dis[alld]).astype(np.float32)

    # global row in the AllGather output for each source node
    yg_row = (alls // NLOC) * NPAD + (alls % NLOC)

    # group edges by (core, dst tile)
    core = alld // NLOC
    loc = alld % NLOC
    t = loc // P
    d = loc % P  # local offset within dst tile
    counts = np.zeros((NCORES, MT), dtype=np.int64)
    np.add.at(counts, (core, t), 1)
    # chunk 0 of each tile = self loops (plain DMA from local Y); rest edges
    c_list = [1 + max(1, int(np.ceil(counts[:, tt].max() / P))) for tt in range(MT)]
    off = np.zeros(MT, dtype=np.int64)
    off[1:] = np.cumsum(c_list)[:-1]
    nch = int(sum(c_list))

    gidx = np.zeros((NCORES, P, nch), dtype=np.int32)
    oh = np.zeros((NCORES, nch * P, P), dtype=np.float32)

    # self chunks: diag(dis[d]^2) per (core, tile)
    for cc in range(NCORES):
        for tt in range(MT):
            nreal = min(P, NLOC - tt * P)
            gl = cc * NLOC + tt * P + np.arange(nreal)
            ch = off[tt]
            oh[cc, ch * P + np.arange(nreal), np.arange(nreal)] = (
                dis[gl] * dis[gl]
            )

    order = np.lexsort((alls, t, core))  # stable ordering by (core, tile)
    so_core, so_t, so_d = core[order], t[order], d[order]
    so_w, so_yg = w[order], yg_row[order]
    grp = so_core * MT + so_t
    start = np.zeros(NCORES * MT + 1, dtype=np.int64)
    np.add.at(start, grp + 1, 1)
    start = np.cumsum(start)
    pos = np.arange(len(order)) - start[grp]
    chunk = off[so_t] + 1 + pos // P
    row = pos % P
    gidx[so_core, row, chunk] = so_yg.astype(np.int32)
    oh[so_core, chunk * P + row, so_d] = so_w

    # int16 index stream for dma_gather: edge-chunk ids exclude self chunks
    n_self_before = np.zeros(nch, dtype=np.int64)
    for tt in range(MT):
        n_self_before[off[tt]:] += 0  # placeholder
    # chunk -> edge-chunk id: subtract #self chunks with index <= chunk
    selfmask = np.zeros(nch, dtype=np.int64)
    selfmask[off] = 1
    ech_of = np.cumsum(selfmask) - 1  # for self chunks: id of tile
    ech_map = np.arange(nch) - np.cumsum(selfmask)  # edge-chunk id (c>0)
    nech = nch - MT
    ni_tot = nech * P
    idx16 = np.zeros((NCORES, 128, ni_tot // 16), dtype=np.int16)
    e_ch = ech_map[chunk]  # edge-chunk id per sorted edge
    i_flat = e_ch * P + row
    p16 = i_flat % 16
    c16 = i_flat // 16
    for cc in range(NCORES):
        m = so_core == cc
        a = np.zeros((16, ni_tot // 16), np.int16)
        a[p16[m], c16[m]] = so_yg[m].astype(np.int16)
        idx16[cc] = np.tile(a, (8, 1))
    return nch, c_list, gidx, oh.astype(BF), idx16, nech


# --------------------------------------------------------------- bass build

def _build(nch, c_list, nech, stage=4, fake_ag=False):
    """Emit the SPMD Bass program. stage: 1=proj only, 2=+1 hop,
    3=+3 hops, 4=full (MLP+LN2). For stage<4 the output is the ct
    (feature-major) tiles of the last computed hop, [768, NPAD] f32."""
    nc = bacc.Bacc(
        "TRN2", target_bir_lowering=False, debug=False, num_devices=NCORES,
        num_swdge_queues=4,
    )
    dp = nc.declare_dram_parameter
    xT = dp("xT", [P, MT * IN_F], BF16, isOutput=False)
    projW = dp("projW", [IN_F, H], BF16, isOutput=False)
    gcnW = dp("gcnW", [HOPS * H, H], BF16, isOutput=False)
    w1 = dp("w1", [P, HK * (HOPS + 1) * H], BF16, isOutput=False)
    w2 = dp("w2", [H, H], BF16, isOutput=False)
    pbias = dp("pbias", [P, H], F32, isOutput=False)
    ln1g = dp("ln1g", [P, H], F32, isOutput=False)
    ln1b = dp("ln1b", [P, H], F32, isOutput=False)
    gbcol = dp("gbcol", [P, HOPS * HK], F32, isOutput=False)  # per-partition
    b1col = dp("b1col", [P, HK], F32, isOutput=False)
    b2 = dp("b2", [P, H], F32, isOutput=False)
    ln2g = dp("ln2g", [P, H], F32, isOutput=False)
    ln2b = dp("ln2b", [P, H], F32, isOutput=False)
    gidx = dp("gidx", [128, (nech * P) // 16], mybir.dt.int16, isOutput=False)
    ohw = dp("oh", [P, nch * P], BF16, isOutput=False)

    nhop = 0 if stage <= 1 else (1 if stage == 2 else HOPS)
    if stage >= 4:
        out = dp("out", [NPAD, H], F32, isOutput=True)
    else:
        out = dp("out", [H, NPAD], F32, isOutput=True)

    off = np.zeros(MT, dtype=np.int64)
    off[1:] = np.cumsum(c_list)[:-1]

    with tile.TileContext(nc) as tc:
        import contextlib

        with contextlib.ExitStack() as ctx:
            dram = ctx.enter_context(tc.tile_pool(name="dram", bufs=1, space="DRAM"))
            cat = ctx.enter_context(tc.tile_pool(name="cat", bufs=1))
            cst = ctx.enter_context(tc.tile_pool(name="cst", bufs=1))

            # persistent feature-major activation tiles
            ct = [cat.tile([P, NPAD], BF16, name=f"ct{i}") for i in range(CK)]

            idx_sb = cst.tile([128, (nech * P) // 16], mybir.dt.int16)
            nc.sync.dma_start(out=idx_sb[:], in_=gidx[:])
            gb_sb = cst.tile([P, HOPS * HK], F32)
            nc.sync.dma_start(out=gb_sb[:], in_=gbcol[:])
            ident = cst.tile([P, P], BF16)
            make_identity(nc, ident[:])
            eps_t = cst.tile([P, 1], F32)
            nc.gpsimd.memset(eps_t[:], LN_EPS)

            # ---------------- proj + LN1 -> ct[0..5] (via PE transpose)
            with tc.tile_pool(name="proj", bufs=1) as pp, \
                    tc.tile_pool(name="psum_pj", bufs=1, space="PSUM") as psum:
                pw = [pp.tile([P, H], BF16, name=f"pw{k}") for k in range(INK)]
                for k in range(INK):
                    nc.sync.dma_start(out=pw[k][:], in_=projW[k * P:(k + 1) * P, :])
                pb_sb = pp.tile([P, H], F32)
                l1g_sb = pp.tile([P, H], F32)
                l1b_sb = pp.tile([P, H], F32)
                nc.sync.dma_start(out=pb_sb[:], in_=pbias[:])
                nc.sync.dma_start(out=l1g_sb[:], in_=ln1g[:])
                nc.sync.dma_start(out=l1b_sb[:], in_=ln1b[:])

                for m in range(MT):
                    ms = slice(m * P, (m + 1) * P)
                    xs = pp.tile([P, INK, P], BF16, tag="xslab", bufs=3)
                    nc.sync.dma_start(
                        out=xs[:],
                        in_=xT[:, m * IN_F:(m + 1) * IN_F].rearrange(
                            "p (k n) -> p k n", n=P
                        ),
                    )
                    ps = psum.tile([P, H], F32, tag="pj", bufs=2)
                    for n0, nn in NSL:
                        for k in range(INK):
                            nc.tensor.matmul(
                                out=ps[:, n0:n0 + nn],
                                lhsT=xs[:, k, :],
                                rhs=pw[k][:, n0:n0 + nn],
                                start=(k == 0),
                                stop=(k == INK - 1),
                            )
                    # LN1 over features (free dim), node-major
                    t0 = pp.tile([P, H], F32, tag="t0", bufs=2)
                    nc.vector.tensor_add(out=t0[:], in0=ps[:], in1=pb_sb[:])
                    mu = pp.tile([P, 1], F32, tag="mu", bufs=2)
                    nc.vector.reduce_sum(out=mu[:], in_=t0[:], axis=mybir.AxisListType.X)
                    nc.scalar.mul(out=mu[:], in_=mu[:], mul=1.0 / H)
                    nc.vector.tensor_scalar_sub(out=t0[:], in0=t0[:], scalar1=mu[:, :1])
                    sq = pp.tile([P, H], F32, tag="sq", bufs=2)
                    nc.vector.tensor_mul(out=sq[:], in0=t0[:], in1=t0[:])
                    var = pp.tile([P, 1], F32, tag="var", bufs=2)
                    nc.vector.reduce_sum(out=var[:], in_=sq[:], axis=mybir.AxisListType.X)
                    rs = pp.tile([P, 1], F32, tag="rs", bufs=2)
                    nc.scalar.activation(
                        out=rs[:], in_=var[:],
                        func=mybir.ActivationFunctionType.Sqrt,
                        scale=1.0 / H, bias=eps_t[:, :1],
                    )
                    nc.vector.reciprocal(out=rs[:], in_=rs[:])
                    nc.vector.tensor_scalar_mul(out=t0[:], in0=t0[:], scalar1=rs[:, :1])
                    nc.vector.tensor_mul(out=t0[:], in0=t0[:], in1=l1g_sb[:])
                    h0 = pp.tile([P, H], BF16, tag="h0", bufs=2)
                    nc.vector.tensor_add(out=h0[:], in0=t0[:], in1=l1b_sb[:])
                    # transpose 6 blocks -> ct[f][:, m]
                    for f in range(HK):
                        tp = psum.tile([P, P], BF16, tag="tp", bufs=2)
                        nc.tensor.transpose(
                            out=tp[:], in_=h0[:, f * P:(f + 1) * P], identity=ident[:]
                        )
                        nc.vector.tensor_copy(out=ct[f][:, ms], in_=tp[:])

            # ---------------- hops
            for k in range(nhop):
                hp = tc.tile_pool(name=f"hop{k}", bufs=1)
                with hp as hpool, \
                        tc.tile_pool(name=f"psum_h{k}", bufs=1, space="PSUM") as psum:
                    gw = [hpool.tile([P, H], BF16, name=f"gw{k}_{f}") for f in range(HK)]
                    for f in range(HK):
                        nc.sync.dma_start(
                            out=gw[f][:], in_=gcnW[k * H + f * P:k * H + (f + 1) * P, :]
                        )
                    ybig = hpool.tile([P, MT * H], BF16)
                    for m in range(MT):
                        ms = slice(m * P, (m + 1) * P)
                        ps = psum.tile([P, H], F32, tag="y", bufs=2)
                        for n0, nn in NSL:
                            for f in range(HK):
                                nc.tensor.matmul(
                                    out=ps[:, n0:n0 + nn],
                                    lhsT=ct[6 * k + f][:, ms],
                                    rhs=gw[f][:, n0:n0 + nn],
                                    start=(f == 0),
                                    stop=(f == HK - 1),
                                )
                        nc.vector.tensor_copy(out=ybig[:, m * H:(m + 1) * H], in_=ps[:])
                    agin = dram.tile([NPAD, H], BF16, name=f"agin{k}")
                    nc.sync.dma_start(
                        out=agin.rearrange("(m p) h -> p m h", p=P),
                        in_=ybig[:].rearrange("p (m h) -> p m h", h=H),
                    )
                    yg = dram.tile(
                        [NCORES * NPAD, H], BF16, addr_space="Shared", name=f"yg{k}"
                    )
                    if fake_ag:
                        # timing-proxy only: local copy standing in for the
                        # AllGather (the sim's collective model is ~12x
                        # pessimistic for intra-chip groups)
                        nc.gpsimd.dma_start(out=yg[0:NPAD, :], in_=agin[:])
                    else:
                        nc.gpsimd.collective_compute(
                            "AllGather",
                            mybir.AluOpType.bypass,
                            ins=[agin.opt()],
                            outs=[yg.opt()],
                            replica_groups=[list(range(NCORES))],
                        )
                    # warm all 8 SWDGE lanes with 1-dep dummy reads of yg
                    for dlane in range(8):
                        dmy = hpool.tile([2, 4], BF16, tag=f"dmy{dlane}", bufs=1)
                        nc.gpsimd.dma_start(out=dmy[:], in_=yg[dlane * 2:dlane * 2 + 2, 0:4])
                    if k == 0:
                        # warm lanes on the idx region too (SBUF->SBUF tiny)
                        for dlane in range(8):
                            dmi = hpool.tile([2, 1], I32, tag=f"dmi{dlane}", bufs=1)
                            nc.gpsimd.dma_start(out=dmi[:], in_=idx_sb[dlane:dlane + 2, 0:1])

                    # flat chunk walk: grouped dma_gathers + batched onehots
                    nch_tot = int(sum(c_list))
                    oh_tiles = {}
                    g_tiles = {}
                    ech = 0  # running edge-chunk id
                    for t in range(MT):
                        ts = slice(t * P, (t + 1) * P)
                        pa = psum.tile([P, 512], F32, tag="sca", bufs=2)
                        pb_ = psum.tile([P, 256], F32, tag="scb", bufs=2)
                        for c in range(c_list[t]):
                            ch = int(off[t]) + c
                            if c == 0:
                                # self-loop chunk: local contiguous rows
                                g = hpool.tile([P, H], BF16, tag="gs", bufs=3,
                                               name=f"gs{k}_{t}")
                                nc.sync.dma_start(
                                    out=g[:], in_=agin[t * P:(t + 1) * P, :]
                                )
                                gsl = g[:, :]
                            else:
                                gg, gj = ech // GRP, ech % GRP
                                if gj == 0:
                                    ng = min(GRP, nech - gg * GRP)
                                    gt = hpool.tile([P, ng, H], BF16, tag="g",
                                                    bufs=2, name=f"g{k}_{gg}")
                                    nc.gpsimd.dma_gather(
                                        out_ap=gt[:],
                                        in_ap=yg[:],
                                        idxs_ap=idx_sb[
                                            :, gg * GRP * 8:(gg * GRP + ng) * 8
                                        ],
                                        num_idxs=ng * P,
                                        num_idxs_reg=ng * P,
                                        elem_size=H,
                                        queue_num=gg % 4,
                                    )
                                    g_tiles[gg] = gt
                                gsl = g_tiles[gg][:, gj, :]
                                ech += 1
                            og, oj = ch // OB, ch % OB
                            if oj == 0:
                                no = min(OB, nch_tot - og * OB)
                                oh_t = hpool.tile([P, no, P], BF16, tag="oh", bufs=3,
                                                  name=f"oh{k}_{og}")
                                nc.sync.dma_start(
                                    out=oh_t[:],
                                    in_=ohw[
                                        :, og * OB * P:(og * OB + no) * P
                                    ].rearrange("p (c m) -> p c m", m=P),
                                )
                                oh_tiles[og] = oh_t
                            oh_t = oh_tiles[og]
                            first, last = (c == 0), (c == c_list[t] - 1)
                            for f in range(HK):
                                dst = pa[:, (f % 4) * P:(f % 4 + 1) * P] if f < 4 else \
                                    pb_[:, (f - 4) * P:(f - 3) * P]
                                nc.tensor.matmul(
                                    out=dst,
                                    lhsT=gsl[:, f * P:(f + 1) * P],
                                    rhs=oh_t[:, oj, :],
                                    start=first and f in (0, 4),
                                    stop=last and f in (3, 5),
                                )
                        for f in range(HK):
                            src = pa[:, (f % 4) * P:(f % 4 + 1) * P] if f < 4 else \
                                pb_[:, (f - 4) * P:(f - 3) * P]
                            nc.scalar.activation(
                                out=ct[6 * (k + 1) + f][:, ts],
                                in_=src,
                                func=mybir.ActivationFunctionType.Gelu,
                                bias=gb_sb[:, k * HK + f:k * HK + f + 1],
                            )

            if stage < 4:
                # dump last hop's ct tiles as [H, NPAD] f32
                with tc.tile_pool(name="dump", bufs=1) as dpool:
                    for f in range(HK):
                        df = dpool.tile([P, NPAD], F32, tag="df", bufs=2)
                        nc.vector.tensor_copy(out=df[:], in_=ct[6 * nhop + f][:])
                        nc.sync.dma_start(out=out[f * P:(f + 1) * P, :], in_=df[:])

            if stage >= 4:
                # ---------------- MLP + LN2
                with tc.tile_pool(name="mlp", bufs=1) as mp, \
                        tc.tile_pool(name="psum_mlp", bufs=1, space="PSUM") as psum:
                    w2t = [mp.tile([P, H], BF16, name=f"w2t{f}") for f in range(HK)]
                    for f in range(HK):
                        nc.sync.dma_start(out=w2t[f][:], in_=w2[f * P:(f + 1) * P, :])
                    b1_sb = mp.tile([P, HK], F32)
                    nc.sync.dma_start(out=b1_sb[:], in_=b1col[:])
                    b2_sb = mp.tile([P, H], F32)
                    l2g_sb = mp.tile([P, H], F32)
                    l2b_sb = mp.tile([P, H], F32)
                    nc.sync.dma_start(out=b2_sb[:], in_=b2[:])
                    nc.sync.dma_start(out=l2g_sb[:], in_=ln2g[:])
                    nc.sync.dma_start(out=l2b_sb[:], in_=ln2b[:])
                    for n in range(5):  # 512-wide node chunks
                        ns = slice(n * 512, (n + 1) * 512)
                        zt = [
                            mp.tile([P, 512], BF16, tag=f"zt{f}", bufs=2, name=f"zt{f}")
                            for f in range(HK)
                        ]
                        for f in range(HK):
                            ws = mp.tile([P, CK, P], BF16, tag="w1s", bufs=2)
                            nc.sync.dma_start(
                                out=ws[:],
                                in_=w1[:, f * CK * P:(f + 1) * CK * P].rearrange(
                                    "p (k n) -> p k n", n=P
                                ),
                            )
                            pz = psum.tile([P, 512], F32, tag="z", bufs=2)
                            for kk in range(CK):
                                nc.tensor.matmul(
                                    out=pz[:],
                                    lhsT=ws[:, kk, :],
                                    rhs=ct[kk][:, ns],
                                    start=(kk == 0),
                                    stop=(kk == CK - 1),
                                )
                            nc.scalar.activation(
                                out=zt[f][:],
                                in_=pz[:],
                                func=mybir.ActivationFunctionType.Gelu,
                                bias=b1_sb[:, f:f + 1],
                            )
                        for mm in range(4):
                            m = n * 4 + mm
                            po = psum.tile([P, H], F32, tag="o", bufs=2)
                            for n0, nn in NSL:
                                for f in range(HK):
                                    nc.tensor.matmul(
                                        out=po[:, n0:n0 + nn],
                                        lhsT=zt[f][:, mm * P:(mm + 1) * P],
                                        rhs=w2t[f][:, n0:n0 + nn],
                                        start=(f == 0),
                                        stop=(f == HK - 1),
                                    )
                            t0 = mp.tile([P, H], F32, tag="t0", bufs=2)
                            nc.vector.tensor_add(out=t0[:], in0=po[:], in1=b2_sb[:])
                            mu = mp.tile([P, 1], F32, tag="mu", bufs=2)
                            nc.vector.reduce_sum(
                                out=mu[:], in_=t0[:], axis=mybir.AxisListType.X
                            )
                            nc.scalar.mul(out=mu[:], in_=mu[:], mul=1.0 / H)
                            nc.vector.tensor_scalar_sub(
                                out=t0[:], in0=t0[:], scalar1=mu[:, :1]
                            )
                            sq = mp.tile([P, H], F32, tag="sq", bufs=2)
                            nc.vector.tensor_mul(out=sq[:], in0=t0[:], in1=t0[:])
                            var = mp.tile([P, 1], F32, tag="var", bufs=2)
                            nc.vector.reduce_sum(
                                out=var[:], in_=sq[:], axis=mybir.AxisListType.X
                            )
                            rs = mp.tile([P, 1], F32, tag="rs", bufs=2)
                            nc.scalar.activation(
                                out=rs[:], in_=var[:],
                                func=mybir.ActivationFunctionType.Sqrt,
                                scale=1.0 / H, bias=eps_t[:, :1],
                            )
                            nc.vector.reciprocal(out=rs[:], in_=rs[:])
                            nc.vector.tensor_scalar_mul(
                                out=t0[:], in0=t0[:], scalar1=rs[:, :1]
                            )
                            nc.vector.tensor_mul(out=t0[:], in0=t0[:], in1=l2g_sb[:])
                            ot = mp.tile([P, H], F32, tag="ot", bufs=3)
                            nc.vector.tensor_add(out=ot[:], in0=t0[:], in1=l2b_sb[:])
                            nc.sync.dma_start(out=out[m * P:(m + 1) * P, :], in_=ot[:])
    nc.compile()
    return nc


def check_waits(nc, limit=1):
    """Return list of DMA-queue instructions exceeding the wait limit."""
    bad = []
    for f in nc.m.functions:
        for bb in f.blocks:
            for ins in bb.instructions:
                tn = type(ins).__name__
                if tn not in ("InstDMACopy", "InstDmaTransposeAnt"):
                    continue
                si = ins.sync_info
                if len(si.on_wait) > limit:
                    bad.append(
                        (ins.name, tn, str(ins.engine),
                         [(w.ant_name, w.wait_value) for w in si.on_wait])
                    )
    return bad


# ------------------------------------------------------------- entry point

def _in_maps(inputs, nch, gidx, oh, idx16):
    x = np.asarray(inputs["x"], dtype=np.float32)
    bcast = lambda v: np.broadcast_to(
        np.asarray(v, np.float32), (P, H)
    ).copy()
    gb = np.asarray(inputs["gcn_b"], np.float32)  # [HOPS, H]
    gbcol = np.zeros((P, HOPS * HK), np.float32)
    for k in range(HOPS):
        for f in range(HK):
            gbcol[:, k * HK + f] = gb[k, f * P:(f + 1) * P]
    b1 = np.asarray(inputs["mlp_b1"], np.float32)
    b1col = np.zeros((P, HK), np.float32)
    for f in range(HK):
        b1col[:, f] = b1[f * P:(f + 1) * P]
    w1 = np.asarray(inputs["mlp_w1"], np.float32)  # [3072, 768]
    w1p = np.zeros((P, HK * (HOPS + 1) * H), np.float32)
    for f in range(HK):
        blk = w1[:, f * P:(f + 1) * P]  # [3072, 128]
        w1p[:, f * CK * P:(f + 1) * CK * P] = (
            blk.reshape(CK, P, P).transpose(1, 0, 2).reshape(P, CK * P)
        )
    common = {
        "projW": np.asarray(inputs["proj_w"], np.float32).astype(BF),
        "gcnW": np.asarray(inputs["gcn_w"], np.float32).reshape(HOPS * H, H).astype(BF),
        "w1": w1p.astype(BF),
        "w2": np.asarray(inputs["mlp_w2"], np.float32).astype(BF),
        "pbias": bcast(inputs["proj_b"]),
        "ln1g": bcast(inputs["ln1_g"]),
        "ln1b": bcast(inputs["ln1_b"]),
        "gbcol": gbcol,
        "b1col": b1col,
        "b2": bcast(inputs["mlp_b2"]),
        "ln2g": bcast(inputs["ln2_g"]),
        "ln2b": bcast(inputs["ln2_b"]),
    }
    maps = []
    for c in range(NCORES):
        xc = np.zeros((NPAD, IN_F), np.float32)
        xc[:NLOC] = x[c * NLOC:(c + 1) * NLOC]
        # pack: xp[p, m*IN_F + k*128 + n] = x[m*128+n, k*128+p]
        xp = (
            xc.reshape(MT, P, INK, P)      # [m, n, k, p]
            .transpose(3, 0, 2, 1)          # [p, m, k, n]
            .reshape(P, MT * IN_F)
        )
        ohp = (
            oh[c].astype(np.float32).reshape(-1, P, P)  # [ch, p, d]
            .transpose(1, 0, 2)                          # [p, ch, d]
            .reshape(P, -1)
        )
        m = dict(common)
        m["xT"] = xp.astype(BF)
        m["gidx"] = idx16[c]
        m["oh"] = ohp.astype(BF)
        maps.append(m)
    return maps


def kernel(**inputs):
    nch, c_list, gidx, oh, idx16, nech = _prep(np.asarray(inputs["edge_index"]))
    nc = _build(nch, c_list, nech, stage=4)
    maps = _in_maps(inputs, nch, gidx, oh, idx16)
    res = run_bass_kernel_spmd(nc, maps, list(range(NCORES)))
    outs = [res.results[c]["out"][:NLOC] for c in range(NCORES)]
    return np.concatenate(outs, axis=0).astype(np.float32)



# revision 3
# speedup vs baseline: 18.8378x; 18.8378x over previous
"""GCN context-paper kernel for 8 trn2 NeuronCores (SPMD via bass/Tile).

Model (see reference): proj+LN -> 3x GCNConv(+self loops, sym-norm) with
GELU -> concat(4 hops) -> MLP(GELU) -> LN.

Sharding: nodes partitioned across 8 cores (2500/core, padded to 2560).
Per hop: each core computes Y = h @ W for its nodes, AllGathers Y (bf16),
then builds its nodes' aggregation with indirect row-gathers of Y plus
one-hot matmuls on the tensor engine (edge weights folded into the
one-hot values, self-loops folded into the edge list).

Layout strategy: activations are kept feature-major ("ct" tiles,
[128 feat, 2560 nodes]) which the scatter matmul produces directly and
all lhsT uses consume directly; only the proj output needs PE transposes.

DMA discipline: every DMA-queue instruction must end up with at most ONE
semaphore wait (hardware struct limit). Hence: DMA destinations in SBUF
are either fresh tiles or have engine-op (not DMA) prior writers; DMA
sources are external inputs or covered by dummy lane-warming DMAs
(collective output).
"""

import numpy as np
import ml_dtypes

import concourse.bass as bass
import concourse.bacc as bacc
import concourse.mybir as mybir
import concourse.tile as tile
from concourse.bass_utils import run_bass_kernel_spmd
from concourse.masks import make_identity

# problem constants (hardcoded per contract)
N, E, IN_F, H, HOPS = 20000, 100000, 1536, 768, 3
LN_EPS = 1e-5
NCORES = 8
NLOC = N // NCORES            # 2500 real nodes per core
P = 128
MT = 20                       # node tiles per core
NPAD = MT * P                 # 2560 padded nodes per core
HK = H // P                   # 6 feature tiles
INK = IN_F // P               # 12
CK = (HOPS + 1) * H // P      # 24 cat feature tiles
NSL = ((0, 512), (512, 256))  # N-dim slices for 768-wide outputs
OB = 8                        # chunks per one-hot load
GRP = 8                       # edge chunks per dma_gather

F32 = mybir.dt.float32
BF16 = mybir.dt.bfloat16
I32 = mybir.dt.int32
BF = ml_dtypes.bfloat16

# sim-only override: CoreSim requires one SWDGE queue per DMA semaphore;
# set to 0 to force all dma_gathers onto queue 0 when simulating.
GATHER_QUEUE = None


# ---------------------------------------------------------------- host prep

def _prep(edge_index):
    """Host preprocessing: normalization, edge sorting, per-core chunk
    tables (gather indices + one-hot weight blocks)."""
    src = np.asarray(edge_index[0], dtype=np.int64)
    dst = np.asarray(edge_index[1], dtype=np.int64)
    deg = np.bincount(dst, minlength=N).astype(np.float64) + 1.0
    dis = 1.0 / np.sqrt(deg)

    # real edges only; self loops become a dedicated per-tile identity chunk
    alls, alld = src, dst
    w = (dis[alls] * dis[alld]).astype(np.float32)

    # global row in the AllGather output for each source node
    yg_row = (alls // NLOC) * NPAD + (alls % NLOC)

    # group edges by (core, dst tile)
    core = alld // NLOC
    loc = alld % NLOC
    t = loc // P
    d = loc % P  # local offset within dst tile
    counts = np.zeros((NCORES, MT), dtype=np.int64)
    np.add.at(counts, (core, t), 1)
    # chunk 0 of each tile = self loops (plain DMA from local Y); rest edges
    c_list = [1 + max(1, int(np.ceil(counts[:, tt].max() / P))) for tt in range(MT)]
    off = np.zeros(MT, dtype=np.int64)
    off[1:] = np.cumsum(c_list)[:-1]
    nch = int(sum(c_list))

    gidx = np.zeros((NCORES, P, nch), dtype=np.int32)
    oh = np.zeros((NCORES, nch * P, P), dtype=np.float32)

    # self chunks: diag(dis[d]^2) per (core, tile)
    for cc in range(NCORES):
        for tt in range(MT):
            nreal = min(P, NLOC - tt * P)
            gl = cc * NLOC + tt * P + np.arange(nreal)
            ch = off[tt]
            oh[cc, ch * P + np.arange(nreal), np.arange(nreal)] = (
                dis[gl] * dis[gl]
            )

    order = np.lexsort((alls, t, core))  # stable ordering by (core, tile)
    so_core, so_t, so_d = core[order], t[order], d[order]
    so_w, so_yg = w[order], yg_row[order]
    grp = so_core * MT + so_t
    start = np.zeros(NCORES * MT + 1, dtype=np.int64)
    np.add.at(start, grp + 1, 1)
    start = np.cumsum(start)
    pos = np.arange(len(order)) - start[grp]
    chunk = off[so_t] + 1 + pos // P
    row = pos % P
    gidx[so_core, row, chunk] = so_yg.astype(np.int32)
    oh[so_core, chunk * P + row, so_d] = so_w

    # int16 index stream for dma_gather: edge-chunk ids exclude self chunks
    n_self_before = np.zeros(nch, dtype=np.int64)
    for tt in range(MT):
        n_self_before[off[tt]:] += 0  # placeholder
    # chunk -> edge-chunk id: subtract #self chunks with index <= chunk
    selfmask = np.zeros(nch, dtype=np.int64)
    selfmask[off] = 1
    ech_of = np.cumsum(selfmask) - 1  # for self chunks: id of tile
    ech_map = np.arange(nch) - np.cumsum(selfmask)  # edge-chunk id (c>0)
    nech = nch - MT
    ni_tot = nech * P
    idx16 = np.zeros((NCORES, 128, ni_tot // 16), dtype=np.int16)
    e_ch = ech_map[chunk]  # edge-chunk id per sorted edge
    i_flat = e_ch * P + row
    p16 = i_flat % 16
    c16 = i_flat // 16
    for cc in range(NCORES):
        m = so_core == cc
        a = np.zeros((16, ni_tot // 16), np.int16)
        a[p16[m], c16[m]] = so_yg[m].astype(np.int16)
        idx16[cc] = np.tile(a, (8, 1))
    return nch, c_list, gidx, oh.astype(BF), idx16, nech


# --------------------------------------------------------------- bass build

def _build(nch, c_list, nech, stage=4, fake_ag=False):
    """Emit the SPMD Bass program. stage: 1=proj only, 2=+1 hop,
    3=+3 hops, 4=full (MLP+LN2). For stage<4 the output is the ct
    (feature-major) tiles of the last computed hop, [768, NPAD] f32."""
    nc = bacc.Bacc(
        "TRN2", target_bir_lowering=False, debug=False, num_devices=NCORES,
        num_swdge_queues=4,
    )
    dp = nc.declare_dram_parameter
    xT = dp("xT", [P, MT * IN_F], BF16, isOutput=False)
    projW = dp("projW", [IN_F, H], BF16, isOutput=False)
    gcnW = dp("gcnW", [HOPS * H, H], BF16, isOutput=False)
    w1 = dp("w1", [P, HK * (HOPS + 1) * H], BF16, isOutput=False)
    w2 = dp("w2", [H, H], BF16, isOutput=False)
    pbias = dp("pbias", [P, H], F32, isOutput=False)
    ln1g = dp("ln1g", [P, H], F32, isOutput=False)
    ln1b = dp("ln1b", [P, H], F32, isOutput=False)
    gbcol = dp("gbcol", [P, HOPS * HK], F32, isOutput=False)  # per-partition
    b1col = dp("b1col", [P, HK], F32, isOutput=False)
    b2 = dp("b2", [P, H], F32, isOutput=False)
    ln2g = dp("ln2g", [P, H], F32, isOutput=False)
    ln2b = dp("ln2b", [P, H], F32, isOutput=False)
    gidx = dp("gidx", [128, (nech * P) // 16], mybir.dt.int16, isOutput=False)
    ohw = dp("oh", [P, nch * P], BF16, isOutput=False)

    nhop = 0 if stage <= 1 else (1 if stage == 2 else HOPS)
    if stage >= 4:
        out = dp("out", [NPAD, H], F32, isOutput=True)
    else:
        out = dp("out", [H, NPAD], F32, isOutput=True)

    off = np.zeros(MT, dtype=np.int64)
    off[1:] = np.cumsum(c_list)[:-1]

    with tile.TileContext(nc) as tc:
        import contextlib

        with contextlib.ExitStack() as ctx:
            dram = ctx.enter_context(tc.tile_pool(name="dram", bufs=1, space="DRAM"))
            cat = ctx.enter_context(tc.tile_pool(name="cat", bufs=1))
            cst = ctx.enter_context(tc.tile_pool(name="cst", bufs=1))

            # persistent feature-major activation tiles
            ct = [cat.tile([P, NPAD], BF16, name=f"ct{i}") for i in range(CK)]

            idx_sb = cst.tile([128, (nech * P) // 16], mybir.dt.int16)
            nc.sync.dma_start(out=idx_sb[:], in_=gidx[:])
            gb_sb = cst.tile([P, HOPS * HK], F32)
            nc.sync.dma_start(out=gb_sb[:], in_=gbcol[:])
            ident = cst.tile([P, P], BF16)
            make_identity(nc, ident[:])
            eps_t = cst.tile([P, 1], F32)
            nc.gpsimd.memset(eps_t[:], LN_EPS)

            # ---------------- proj + LN1 -> ct[0..5] (via PE transpose)
            with tc.tile_pool(name="proj", bufs=1) as pp, \
                    tc.tile_pool(name="psum_pj", bufs=1, space="PSUM") as psum:
                pw = [pp.tile([P, H], BF16, name=f"pw{k}") for k in range(INK)]
                for k in range(INK):
                    nc.sync.dma_start(out=pw[k][:], in_=projW[k * P:(k + 1) * P, :])
                pb_sb = pp.tile([P, H], F32)
                l1g_sb = pp.tile([P, H], F32)
                l1b_sb = pp.tile([P, H], F32)
                nc.sync.dma_start(out=pb_sb[:], in_=pbias[:])
                nc.sync.dma_start(out=l1g_sb[:], in_=ln1g[:])
                nc.sync.dma_start(out=l1b_sb[:], in_=ln1b[:])

                for m in range(MT):
                    ms = slice(m * P, (m + 1) * P)
                    xs = pp.tile([P, INK, P], BF16, tag="xslab", bufs=3)
                    nc.sync.dma_start(
                        out=xs[:],
                        in_=xT[:, m * IN_F:(m + 1) * IN_F].rearrange(
                            "p (k n) -> p k n", n=P
                        ),
                    )
                    ps = psum.tile([P, H], F32, tag="pj", bufs=2)
                    for n0, nn in NSL:
                        for k in range(INK):
                            nc.tensor.matmul(
                                out=ps[:, n0:n0 + nn],
                                lhsT=xs[:, k, :],
                                rhs=pw[k][:, n0:n0 + nn],
                                start=(k == 0),
                                stop=(k == INK - 1),
                            )
                    # LN1 over features (free dim), node-major
                    t0 = pp.tile([P, H], F32, tag="t0", bufs=2)
                    nc.vector.tensor_add(out=t0[:], in0=ps[:], in1=pb_sb[:])
                    mu = pp.tile([P, 1], F32, tag="mu", bufs=2)
                    nc.vector.reduce_sum(out=mu[:], in_=t0[:], axis=mybir.AxisListType.X)
                    nc.scalar.mul(out=mu[:], in_=mu[:], mul=1.0 / H)
                    nc.vector.tensor_scalar_sub(out=t0[:], in0=t0[:], scalar1=mu[:, :1])
                    sq = pp.tile([P, H], F32, tag="sq", bufs=2)
                    nc.vector.tensor_mul(out=sq[:], in0=t0[:], in1=t0[:])
                    var = pp.tile([P, 1], F32, tag="var", bufs=2)
                    nc.vector.reduce_sum(out=var[:], in_=sq[:], axis=mybir.AxisListType.X)
                    rs = pp.tile([P, 1], F32, tag="rs", bufs=2)
                    nc.scalar.activation(
                        out=rs[:], in_=var[:],
                        func=mybir.ActivationFunctionType.Sqrt,
                        scale=1.0 / H, bias=eps_t[:, :1],
                    )
                    nc.vector.reciprocal(out=rs[:], in_=rs[:])
                    nc.vector.tensor_scalar_mul(out=t0[:], in0=t0[:], scalar1=rs[:, :1])
                    nc.vector.tensor_mul(out=t0[:], in0=t0[:], in1=l1g_sb[:])
                    h0 = pp.tile([P, H], BF16, tag="h0", bufs=2)
                    nc.vector.tensor_add(out=h0[:], in0=t0[:], in1=l1b_sb[:])
                    # transpose 6 blocks -> ct[f][:, m]
                    for f in range(HK):
                        tp = psum.tile([P, P], BF16, tag="tp", bufs=2)
                        nc.tensor.transpose(
                            out=tp[:], in_=h0[:, f * P:(f + 1) * P], identity=ident[:]
                        )
                        nc.vector.tensor_copy(out=ct[f][:, ms], in_=tp[:])

            # ---------------- hops
            for k in range(nhop):
                hp = tc.tile_pool(name=f"hop{k}", bufs=1)
                with hp as hpool, \
                        tc.tile_pool(name=f"psum_h{k}", bufs=1, space="PSUM") as psum:
                    gw = [hpool.tile([P, H], BF16, name=f"gw{k}_{f}") for f in range(HK)]
                    for f in range(HK):
                        nc.sync.dma_start(
                            out=gw[f][:], in_=gcnW[k * H + f * P:k * H + (f + 1) * P, :]
                        )
                    ybig = hpool.tile([P, MT * H], BF16)
                    for m in range(MT):
                        ms = slice(m * P, (m + 1) * P)
                        ps = psum.tile([P, H], F32, tag="y", bufs=2)
                        for n0, nn in NSL:
                            for f in range(HK):
                                nc.tensor.matmul(
                                    out=ps[:, n0:n0 + nn],
                                    lhsT=ct[6 * k + f][:, ms],
                                    rhs=gw[f][:, n0:n0 + nn],
                                    start=(f == 0),
                                    stop=(f == HK - 1),
                                )
                        nc.vector.tensor_copy(out=ybig[:, m * H:(m + 1) * H], in_=ps[:])
                    agin = dram.tile([NPAD, H], BF16, name=f"agin{k}")
                    nc.sync.dma_start(
                        out=agin.rearrange("(m p) h -> p m h", p=P),
                        in_=ybig[:].rearrange("p (m h) -> p m h", h=H),
                    )
                    yg = dram.tile(
                        [NCORES * NPAD, H], BF16, addr_space="Shared", name=f"yg{k}"
                    )
                    if fake_ag:
                        # timing-proxy only: local copy standing in for the
                        # AllGather (the sim's collective model is ~12x
                        # pessimistic for intra-chip groups)
                        nc.gpsimd.dma_start(out=yg[0:NPAD, :], in_=agin[:])
                    else:
                        nc.gpsimd.collective_compute(
                            "AllGather",
                            mybir.AluOpType.bypass,
                            ins=[agin.opt()],
                            outs=[yg.opt()],
                            replica_groups=[list(range(NCORES))],
                        )
                    # warm all 8 SWDGE lanes with 1-dep dummy reads of yg
                    for dlane in range(8):
                        dmy = hpool.tile([2, 4], BF16, tag=f"dmy{dlane}", bufs=1)
                        nc.gpsimd.dma_start(out=dmy[:], in_=yg[dlane * 2:dlane * 2 + 2, 0:4])
                    if k == 0:
                        # warm lanes on the idx region too (SBUF->SBUF tiny)
                        for dlane in range(8):
                            dmi = hpool.tile([2, 1], I32, tag=f"dmi{dlane}", bufs=1)
                            nc.gpsimd.dma_start(out=dmi[:], in_=idx_sb[dlane:dlane + 2, 0:1])

                    # flat chunk walk: grouped dma_gathers + batched onehots
                    nch_tot = int(sum(c_list))
                    oh_tiles = {}
                    g_tiles = {}
                    ech = 0  # running edge-chunk id
                    for t in range(MT):
                        ts = slice(t * P, (t + 1) * P)
                        pa = psum.tile([P, 512], F32, tag="sca", bufs=2)
                        pb_ = psum.tile([P, 256], F32, tag="scb", bufs=2)
                        for c in range(c_list[t]):
                            ch = int(off[t]) + c
                            if c == 0:
                                # self-loop chunk: local contiguous rows
                                g = hpool.tile([P, H], BF16, tag="gs", bufs=3,
                                               name=f"gs{k}_{t}")
                                nc.sync.dma_start(
                                    out=g[:], in_=agin[t * P:(t + 1) * P, :]
                                )
                                gsl = g[:, :]
                            else:
                                gg, gj = ech // GRP, ech % GRP
                                if gj == 0:
                                    ng = min(GRP, nech - gg * GRP)
                                    gt = hpool.tile([P, ng, H], BF16, tag="g",
                                                    bufs=2, name=f"g{k}_{gg}")
                                    nc.gpsimd.dma_gather(
                                        out_ap=gt[:],
                                        in_ap=yg[:],
                                        idxs_ap=idx_sb[
                                            :, gg * GRP * 8:(gg * GRP + ng) * 8
                                        ],
                                        num_idxs=ng * P,
                                        num_idxs_reg=ng * P,
                                        elem_size=H,
                                        queue_num=(gg % 4) if GATHER_QUEUE is None
                                        else GATHER_QUEUE,
                                    )
                                    g_tiles[gg] = gt
                                gsl = g_tiles[gg][:, gj, :]
                                ech += 1
                            og, oj = ch // OB, ch % OB
                            if oj == 0:
                                no = min(OB, nch_tot - og * OB)
                                oh_t = hpool.tile([P, no, P], BF16, tag="oh", bufs=3,
                                                  name=f"oh{k}_{og}")
                                nc.sync.dma_start(
                                    out=oh_t[:],
                                    in_=ohw[
                                        :, og * OB * P:(og * OB + no) * P
                                    ].rearrange("p (c m) -> p c m", m=P),
                                )
                                oh_tiles[og] = oh_t
                            oh_t = oh_tiles[og]
                            first, last = (c == 0), (c == c_list[t] - 1)
                            for f in range(HK):
                                dst = pa[:, (f % 4) * P:(f % 4 + 1) * P] if f < 4 else \
                                    pb_[:, (f - 4) * P:(f - 3) * P]
                                nc.tensor.matmul(
                                    out=dst,
                                    lhsT=gsl[:, f * P:(f + 1) * P],
                                    rhs=oh_t[:, oj, :],
                                    start=first and f in (0, 4),
                                    stop=last and f in (3, 5),
                                )
                        for f in range(HK):
                            src = pa[:, (f % 4) * P:(f % 4 + 1) * P] if f < 4 else \
                                pb_[:, (f - 4) * P:(f - 3) * P]
                            nc.scalar.activation(
                                out=ct[6 * (k + 1) + f][:, ts],
                                in_=src,
                                func=mybir.ActivationFunctionType.Gelu,
                                bias=gb_sb[:, k * HK + f:k * HK + f + 1],
                            )

            if stage < 4:
                # dump last hop's ct tiles as [H, NPAD] f32
                with tc.tile_pool(name="dump", bufs=1) as dpool:
                    for f in range(HK):
                        df = dpool.tile([P, NPAD], F32, tag="df", bufs=2)
                        nc.vector.tensor_copy(out=df[:], in_=ct[6 * nhop + f][:])
                        nc.sync.dma_start(out=out[f * P:(f + 1) * P, :], in_=df[:])

            if stage >= 4:
                # ---------------- MLP + LN2
                with tc.tile_pool(name="mlp", bufs=1) as mp, \
                        tc.tile_pool(name="psum_mlp", bufs=1, space="PSUM") as psum:
                    w2t = [mp.tile([P, H], BF16, name=f"w2t{f}") for f in range(HK)]
                    for f in range(HK):
                        nc.sync.dma_start(out=w2t[f][:], in_=w2[f * P:(f + 1) * P, :])
                    b1_sb = mp.tile([P, HK], F32)
                    nc.sync.dma_start(out=b1_sb[:], in_=b1col[:])
                    b2_sb = mp.tile([P, H], F32)
                    l2g_sb = mp.tile([P, H], F32)
                    l2b_sb = mp.tile([P, H], F32)
                    nc.sync.dma_start(out=b2_sb[:], in_=b2[:])
                    nc.sync.dma_start(out=l2g_sb[:], in_=ln2g[:])
                    nc.sync.dma_start(out=l2b_sb[:], in_=ln2b[:])
                    for n in range(5):  # 512-wide node chunks
                        ns = slice(n * 512, (n + 1) * 512)
                        zt = [
                            mp.tile([P, 512], BF16, tag=f"zt{f}", bufs=2, name=f"zt{f}")
                            for f in range(HK)
                        ]
                        for f in range(HK):
                            ws = mp.tile([P, CK, P], BF16, tag="w1s", bufs=2)
                            nc.sync.dma_start(
                                out=ws[:],
                                in_=w1[:, f * CK * P:(f + 1) * CK * P].rearrange(
                                    "p (k n) -> p k n", n=P
                                ),
                            )
                            pz = psum.tile([P, 512], F32, tag="z", bufs=2)
                            for kk in range(CK):
                                nc.tensor.matmul(
                                    out=pz[:],
                                    lhsT=ws[:, kk, :],
                                    rhs=ct[kk][:, ns],
                                    start=(kk == 0),
                                    stop=(kk == CK - 1),
                                )
                            nc.scalar.activation(
                                out=zt[f][:],
                                in_=pz[:],
                                func=mybir.ActivationFunctionType.Gelu,
                                bias=b1_sb[:, f:f + 1],
                            )
                        for mm in range(4):
                            m = n * 4 + mm
                            po = psum.tile([P, H], F32, tag="o", bufs=2)
                            for n0, nn in NSL:
                                for f in range(HK):
                                    nc.tensor.matmul(
                                        out=po[:, n0:n0 + nn],
                                        lhsT=zt[f][:, mm * P:(mm + 1) * P],
                                        rhs=w2t[f][:, n0:n0 + nn],
                                        start=(f == 0),
                                        stop=(f == HK - 1),
                                    )
                            t0 = mp.tile([P, H], F32, tag="t0", bufs=2)
                            nc.vector.tensor_add(out=t0[:], in0=po[:], in1=b2_sb[:])
                            mu = mp.tile([P, 1], F32, tag="mu", bufs=2)
                            nc.vector.reduce_sum(
                                out=mu[:], in_=t0[:], axis=mybir.AxisListType.X
                            )
                            nc.scalar.mul(out=mu[:], in_=mu[:], mul=1.0 / H)
                            nc.vector.tensor_scalar_sub(
                                out=t0[:], in0=t0[:], scalar1=mu[:, :1]
                            )
                            sq = mp.tile([P, H], F32, tag="sq", bufs=2)
                            nc.vector.tensor_mul(out=sq[:], in0=t0[:], in1=t0[:])
                            var = mp.tile([P, 1], F32, tag="var", bufs=2)
                            nc.vector.reduce_sum(
                                out=var[:], in_=sq[:], axis=mybir.AxisListType.X
                            )
                            rs = mp.tile([P, 1], F32, tag="rs", bufs=2)
                            nc.scalar.activation(
                                out=rs[:], in_=var[:],
                                func=mybir.ActivationFunctionType.Sqrt,
                                scale=1.0 / H, bias=eps_t[:, :1],
                            )
                            nc.vector.reciprocal(out=rs[:], in_=rs[:])
                            nc.vector.tensor_scalar_mul(
                                out=t0[:], in0=t0[:], scalar1=rs[:, :1]
                            )
                            nc.vector.tensor_mul(out=t0[:], in0=t0[:], in1=l2g_sb[:])
                            ot = mp.tile([P, H], F32, tag="ot", bufs=3)
                            nc.vector.tensor_add(out=ot[:], in0=t0[:], in1=l2b_sb[:])
                            nc.sync.dma_start(out=out[m * P:(m + 1) * P, :], in_=ot[:])
    nc.compile()
    return nc


def check_waits(nc, limit=1):
    """Return list of DMA-queue instructions exceeding the wait limit."""
    bad = []
    for f in nc.m.functions:
        for bb in f.blocks:
            for ins in bb.instructions:
                tn = type(ins).__name__
                if tn not in ("InstDMACopy", "InstDmaTransposeAnt"):
                    continue
                si = ins.sync_info
                if len(si.on_wait) > limit:
                    bad.append(
                        (ins.name, tn, str(ins.engine),
                         [(w.ant_name, w.wait_value) for w in si.on_wait])
                    )
    return bad


# ------------------------------------------------------------- entry point

def _in_maps(inputs, nch, gidx, oh, idx16):
    x = np.asarray(inputs["x"], dtype=np.float32)
    bcast = lambda v: np.broadcast_to(
        np.asarray(v, np.float32), (P, H)
    ).copy()
    gb = np.asarray(inputs["gcn_b"], np.float32)  # [HOPS, H]
    gbcol = np.zeros((P, HOPS * HK), np.float32)
    for k in range(HOPS):
        for f in range(HK):
            gbcol[:, k * HK + f] = gb[k, f * P:(f + 1) * P]
    b1 = np.asarray(inputs["mlp_b1"], np.float32)
    b1col = np.zeros((P, HK), np.float32)
    for f in range(HK):
        b1col[:, f] = b1[f * P:(f + 1) * P]
    w1 = np.asarray(inputs["mlp_w1"], np.float32)  # [3072, 768]
    w1p = np.zeros((P, HK * (HOPS + 1) * H), np.float32)
    for f in range(HK):
        blk = w1[:, f * P:(f + 1) * P]  # [3072, 128]
        w1p[:, f * CK * P:(f + 1) * CK * P] = (
            blk.reshape(CK, P, P).transpose(1, 0, 2).reshape(P, CK * P)
        )
    common = {
        "projW": np.asarray(inputs["proj_w"], np.float32).astype(BF),
        "gcnW": np.asarray(inputs["gcn_w"], np.float32).reshape(HOPS * H, H).astype(BF),
        "w1": w1p.astype(BF),
        "w2": np.asarray(inputs["mlp_w2"], np.float32).astype(BF),
        "pbias": bcast(inputs["proj_b"]),
        "ln1g": bcast(inputs["ln1_g"]),
        "ln1b": bcast(inputs["ln1_b"]),
        "gbcol": gbcol,
        "b1col": b1col,
        "b2": bcast(inputs["mlp_b2"]),
        "ln2g": bcast(inputs["ln2_g"]),
        "ln2b": bcast(inputs["ln2_b"]),
    }
    maps = []
    for c in range(NCORES):
        xc = np.zeros((NPAD, IN_F), np.float32)
        xc[:NLOC] = x[c * NLOC:(c + 1) * NLOC]
        # pack: xp[p, m*IN_F + k*128 + n] = x[m*128+n, k*128+p]
        xp = (
            xc.reshape(MT, P, INK, P)      # [m, n, k, p]
            .transpose(3, 0, 2, 1)          # [p, m, k, n]
            .reshape(P, MT * IN_F)
        )
        ohp = (
            oh[c].astype(np.float32).reshape(-1, P, P)  # [ch, p, d]
            .transpose(1, 0, 2)                          # [p, ch, d]
            .reshape(P, -1)
        )
        m = dict(common)
        m["xT"] = xp.astype(BF)
        m["gidx"] = idx16[c]
        m["oh"] = ohp.astype(BF)
        maps.append(m)
    return maps


def kernel(**inputs):
    nch, c_list, gidx, oh, idx16, nech = _prep(np.asarray(inputs["edge_index"]))
    nc = _build(nch, c_list, nech, stage=4)
    maps = _in_maps(inputs, nch, gidx, oh, idx16)
    res = run_bass_kernel_spmd(nc, maps, list(range(NCORES)))
    outs = [res.results[c]["out"][:NLOC] for c in range(NCORES)]
    return np.concatenate(outs, axis=0).astype(np.float32)

